# revision 23
# baseline (speedup 1.0000x reference)
"""Trainium2 Bass kernel for the 2D circulant transform.

Math: per example b,  out[b] = C_s @ inp[b] @ C_h^T  where C_s/C_h are the
circulant matrices of seq_circ (S=4096) and hidden_circ (H=1024).
Data-parallel over batch: core b handles example b (B == 8 cores).

Stage 1 (S-axis, per 128-col block of H): CRT tree of C_4096 into
cyclic-512 + nega-512 (direct), nega-1024 (two twisted-256 complex
products), nega-2048 (two twisted-512 complex products); all folds are
elementwise on the input and precomputed on host.  fp16 operands/
windows, fp32 PSUM, ScalarE evacuates, VectorE recombines.

Stage 2 (H-axis): C_1024 = cyclic-512 + nega-512 along H.  The
cyclic-512 is further CRT-split into cyclic-256 + nega-256 -- its
operand fold is linear in input H-columns, commutes with the S
transform, and is done for FREE on the host (group-0 operand ships as
[uu|uv]).  c-side chains contract 256 cols into a [c256|n256] PSUM
bank; un-CRT + final H-recombine run on VectorE, output DMA'd fp16 and
upcast on host.

Engine notes (measured on HW):
- GPSIMD tensor ops cost ~1.25us per [128,512] (3.7x DVE) with multi-us
  library-reload warmups, and concurrently-running GPSIMD ops slow DVE
  ops ~4x via SBUF contention -- GPSIMD does DMA issue only.
- Windows that are pure sign flips (w_tpn=-w_tpi, ...) or antiperiodic
  tails (rot_ccn/rot_hcn/rot_hn: w[:,f+M] = -w[:,f]) are generated on
  DVE instead of DMA'd (-1.3MB off the bandwidth-bound startup stream).
- Output DMAs go on the idle sync queue; putting them on the scalar
  queue stalls ScalarE's evacuation stream (head-of-line blocking).
"""
import os
import sys

for _p in ("/opt/trn_rl_repo",):
    if _p not in sys.path and os.path.isdir(_p):
        sys.path.append(_p)

import numpy as np

import concourse.bacc as bacc
import concourse.mybir as mybir
import concourse.tile as tile
from concourse import bass_utils

B, S, H = 8, 4096, 1024
MS, MH = S // 2, H // 2
P = 128
NW = 512
HW2 = 256
F16 = mybir.dt.float16
F32 = mybir.dt.float32
SQ = float(np.sqrt(0.5))

_CACHE = {}

# DMA'd window layout: (name, width).  Sign-flipped partners (w_tpn,
# w_tmn, w2pn, w2mn) are generated on-device from these.  Nega windows
# (rot_ccn/rot_hcn/rot_hn) are antiperiodic (w[:,f+M] = -w[:,f]) so only
# the leading columns ship; the tail is generated on DVE.
WIN_LAYOUT = (
    ("w_tpr", 1408),                                     # seg 0 (first need)
    ("w_tpi", 1408),                                     # seg 1
    ("w_tmr", 1408), ("w_tmi", 1408),                    # seg 2
    ("w2pr", 640), ("w2pi", 640), ("w2mr", 640), ("w2mi", 640),  # seg 3
    ("rot_ccc", 896),                                    # seg 4
    ("rot_hcc", 384),                                    # seg 5
    ("rot_ccn", 896), ("rot_hcn", 384), ("rot_hn", 896),  # anti, own DMAs
)
SEG_SPLIT = (1408, 1408, 2816, 2560, 896, 384)
WIN_TOTAL = sum(w for _, w in WIN_LAYOUT)
GEN_WINS = (("w_tpn", "w_tpi", 1408), ("w_tmn", "w_tmi", 1408),
            ("w2pn", "w2pi", 640), ("w2mn", "w2mi", 640))
# name -> (full_width, shipped, gen_src_lo):  [shipped:full] = -[src:src+len]
ANTI_WINS = {"rot_ccn": (1408, 896, 384), "rot_hcn": (640, 384, 128),
             "rot_hn": (1408, 896, 384)}


def _build():
    nc = bacc.Bacc("TRN2", target_bir_lowering=False, debug=False,
                   num_devices=B)
    d_op = [nc.dram_tensor(f"op{g}", [P, 32 * NW], F16,
                           kind="ExternalInput").ap() for g in range(2)]
    d_win = nc.dram_tensor("wins", [P, WIN_TOTAL], F16,
                           kind="ExternalInput").ap()
    out = nc.dram_tensor("out", [S, H], F16, kind="ExternalOutput").ap()

    with tile.TileContext(nc) as tc:
        with tc.tile_pool(name="const", bufs=1) as cpool, \
             tc.tile_pool(name="work", bufs=1) as wpool, \
             tc.tile_pool(name="io", bufs=2) as iopool, \
             tc.tile_pool(name="ps", bufs=1, space="PSUM") as ppool:
            # seg tiles cover the non-antiperiodic windows contiguously;
            # antiperiodic windows get full-width tiles with partial DMA.
            win = {}
            win_off = {}
            off = 0
            for name, w in WIN_LAYOUT:
                win_off[name] = off
                off += w
            segs = []
            off = 0
            for w in SEG_SPLIT:
                segs.append((off, cpool.tile([P, w], F16, name=f"wseg{off}")))
                off += w
            off = 0
            for name, w in WIN_LAYOUT:
                if name in ANTI_WINS:
                    continue
                for so, st in segs:
                    if so <= off < so + st.shape[1]:
                        win[name] = st[:, off - so:off - so + w]
                        break
                off += w
            for name, (full, ship, src) in ANTI_WINS.items():
                win[name] = cpool.tile([P, full], F16, name=name)[:]
            # device-generated sign-flipped windows (DVE, idle at startup)
            for gname, src, w in GEN_WINS:
                t = cpool.tile([P, w], F16, name=gname)
                win[gname] = t[:]

            def anti_dma(name, eng):
                full, ship, src = ANTI_WINS[name]
                eng.dma_start(win[name][:, 0:ship],
                              d_win[:, win_off[name]:win_off[name] + ship])

            def anti_gen(name):
                full, ship, src = ANTI_WINS[name]
                nc.vector.tensor_scalar_mul(win[name][:, ship:full],
                                            win[name][:, src:src + full - ship],
                                            -1.0)

            opq = [[None] * 8 for _ in range(2)]
            opa = [None, None]
            opc = [None, None]

            def load_opq(g, q, eng):
                for h in range(2):
                    qq = 2 * q + h
                    t = iopool.tile([P, 2 * NW], F16, tag=f"opq{qq}", bufs=2,
                                    name=f"opq{qq}_{g}")
                    eng.dma_start(t[:], d_op[g][:, qq * 2 * NW:
                                                (qq + 1) * 2 * NW])
                    opq[g][qq] = t

            def load_ac_a(g, eng):
                opa[g] = iopool.tile([P, 8 * NW], F16, tag="opa", bufs=1,
                                     name=f"opa_{g}")
                eng.dma_start(opa[g][:], d_op[g][:, 16 * NW:24 * NW])

            def load_ac_c(g, eng):
                opc[g] = iopool.tile([P, 8 * NW], F16, tag="opc", bufs=1,
                                     name=f"opc_{g}")
                eng.dma_start(opc[g][:], d_op[g][:, 24 * NW:32 * NW])

            def seg_dma(eng, i):
                lo = sum(SEG_SPLIT[:i])
                eng.dma_start(segs[i][1][:], d_win[:, lo:lo + SEG_SPLIT[i]])

            # deadline-ordered bulk DMAs across the three DGE queues.
            # gpsimd's SWDGE (~60GB/s) carries only the late-need windows.
            seg_dma(nc.sync, 0)           # w_tpr (first need, alone)
            load_opq(0, 0, nc.scalar)     # d+re
            seg_dma(nc.scalar, 1)         # w_tpi
            load_opq(0, 1, nc.sync)       # d+im
            seg_dma(nc.sync, 2)           # w_tmr | w_tmi
            load_opq(0, 2, nc.scalar)     # d-re
            seg_dma(nc.gpsimd, 4)         # rot_ccc
            anti_dma("rot_ccn", nc.gpsimd)
            seg_dma(nc.scalar, 3)         # w2*
            load_opq(0, 3, nc.sync)       # d-im
            load_ac_a(0, nc.scalar)
            load_ac_c(0, nc.scalar)
            load_opq(1, 0, nc.sync)
            load_opq(1, 1, nc.scalar)
            load_opq(1, 2, nc.sync)
            load_opq(1, 3, nc.scalar)
            seg_dma(nc.gpsimd, 5)         # rot_hcc
            anti_dma("rot_hcn", nc.gpsimd)
            anti_dma("rot_hn", nc.gpsimd)
            load_ac_a(1, nc.sync)
            load_ac_c(1, nc.sync)

            # DVE window gens, ordered by first need
            gen = {g[0]: g for g in GEN_WINS}
            for gname, src, w in (gen["w_tpn"], gen["w_tmn"]):
                nc.vector.tensor_scalar_mul(win[gname], win[src], -1.0)
            anti_gen("rot_ccn")
            for gname, src, w in (gen["w2pn"], gen["w2mn"]):
                nc.vector.tensor_scalar_mul(win[gname], win[src], -1.0)
            anti_gen("rot_hcn")
            anti_gen("rot_hn")

            # stage-1 outputs: yy[side][g][kt][spc], side 0 = yp
            yy = [[[[None] * 4 for _ in range(4)] for _ in range(2)]
                  for _ in range(2)]

            def m_phases(mi):
                g, kt = mi // 4, mi % 4

                def osl(i):
                    if i < 16:
                        buf, j = opq[g][i // 2], i % 2
                    elif i < 24:
                        buf, j = opa[g], i - 16
                    else:
                        buf, j = opc[g], i - 24
                    c0 = j * NW + kt * P
                    return buf[:, c0:c0 + P]

                def chain(tag, mms, n_w=NW):
                    ps = ppool.tile([P, NW], F32, tag=tag, name=f"p_{tag}_{mi}")
                    n = len(mms)
                    for i, (plo, o, wname, d) in enumerate(mms):
                        nc.tensor.matmul(ps[:, plo:plo + n_w], osl(o),
                                         win[wname][:, d:d + n_w],
                                         start=(i == 0), stop=(i == n - 1))
                    return ps

                def evac(name, ps):
                    t = iopool.tile([P, NW], F16, tag=f"{name}e", bufs=2,
                                    name=f"{name}e_{mi}")
                    nc.scalar.mul(t[:], ps[:], 1.0)
                    return t

                def tt(name, a, b, op_, bufs=1, wid=NW, eng=None):
                    t = iopool.tile([P, wid], F16, tag=name, bufs=bufs,
                                    name=f"{name}_{mi}")
                    eng = eng or nc.vector
                    (eng.tensor_add if op_ == "+" else eng.tensor_sub)(
                        t[:], a, b)
                    return t

                dtw = [(-j * P) % 1024 for j in range(4)]

                def phase_e():
                    p_epr = chain("epr",
                                  [(0, j, "w_tpr", dtw[j]) for j in range(4)]
                                  + [(0, 4 + j, "w_tpn", dtw[j]) for j in range(4)])
                    p_epi = chain("epi",
                                  [(0, j, "w_tpi", dtw[j]) for j in range(4)]
                                  + [(0, 4 + j, "w_tpr", dtw[j]) for j in range(4)])
                    e_pr = evac("epr", p_epr)
                    e_pi = evac("epi", p_epi)
                    p_emr = chain("emr",
                                  [(0, 8 + j, "w_tmr", dtw[j]) for j in range(4)]
                                  + [(0, 12 + j, "w_tmn", dtw[j]) for j in range(4)])
                    p_emi = chain("emi",
                                  [(0, 8 + j, "w_tmi", dtw[j]) for j in range(4)]
                                  + [(0, 12 + j, "w_tmr", dtw[j]) for j in range(4)])
                    e_mr = evac("emr", p_emr)
                    e_mi = evac("emi", p_emi)
                    return e_pr, e_pi, e_mr, e_mi

                def phase_r(ev):
                    e_pr, e_pi, e_mr, e_mi = ev
                    ne = [None] * 4
                    ne[0] = tt("ne0", e_pr[:], e_mr[:], "+")
                    ne[2] = tt("ne2", e_pi[:], e_mi[:], "+")
                    dre = tt("dre", e_pr[:], e_mr[:], "-")
                    dim = tt("dim", e_pi[:], e_mi[:], "-")
                    t3 = tt("t3", dre[:], dim[:], "+")
                    t4 = tt("t4", dim[:], dre[:], "-")
                    ne[1] = iopool.tile([P, NW], F16, tag="ne1", bufs=1,
                                        name=f"ne1_{mi}")
                    nc.vector.tensor_scalar_mul(ne[1][:], t3[:], SQ)
                    ne[3] = iopool.tile([P, NW], F16, tag="ne3", bufs=1,
                                        name=f"ne3_{mi}")
                    nc.vector.tensor_scalar_mul(ne[3][:], t4[:], SQ)

                    d2 = (0, 384)
                    p_a0 = chain("a0",
                                 [(0, 16 + j, "w2pr", d2[j]) for j in range(2)]
                                 + [(0, 18 + j, "w2pn", d2[j]) for j in range(2)]
                                 + [(HW2, 20 + j, "w2mr", d2[j]) for j in range(2)]
                                 + [(HW2, 22 + j, "w2mn", d2[j]) for j in range(2)],
                                 n_w=HW2)
                    p_a1 = chain("a1",
                                 [(0, 16 + j, "w2pi", d2[j]) for j in range(2)]
                                 + [(0, 18 + j, "w2pr", d2[j]) for j in range(2)]
                                 + [(HW2, 20 + j, "w2mi", d2[j]) for j in range(2)]
                                 + [(HW2, 22 + j, "w2mr", d2[j]) for j in range(2)],
                                 n_w=HW2)
                    ar0 = evac("a0", p_a0)
                    ar1 = evac("a1", p_a1)
                    aa0 = iopool.tile([P, NW], F16, tag="a0e2", bufs=2,
                                      name=f"a0e2_{mi}")
                    aa1 = iopool.tile([P, NW], F16, tag="a1e2", bufs=2,
                                      name=f"a1e2_{mi}")
                    nc.vector.tensor_add(aa0[:, :HW2], ar0[:, :HW2],
                                         ar0[:, HW2:])
                    nc.vector.tensor_add(aa1[:, :HW2], ar1[:, :HW2],
                                         ar1[:, HW2:])
                    ddre = tt("ddre", ar0[:, :HW2], ar0[:, HW2:], "-", wid=HW2)
                    ddim = tt("ddim", ar1[:, :HW2], ar1[:, HW2:], "-", wid=HW2)
                    t3p = tt("t3p", ddre[:], ddim[:], "+", wid=HW2)
                    t4p = tt("t4p", ddim[:], ddre[:], "-", wid=HW2)
                    nc.vector.tensor_scalar_mul(aa0[:, HW2:], t3p[:], SQ)
                    nc.vector.tensor_scalar_mul(aa1[:, HW2:], t4p[:], SQ)

                    p_c3 = chain("c3", [(0, 24 + k, "rot_ccc", (-k * P) % 512)
                                        for k in range(4)])
                    p_n3 = chain("l3n", [(0, 28 + k, "rot_ccn",
                                          (-k * P) % 1024) for k in range(4)])
                    c3e = evac("c3", p_c3)
                    n3e = evac("l3n", p_n3)

                    e0 = tt("e0", c3e[:], n3e[:], "+")
                    e1 = tt("e1", c3e[:], n3e[:], "-")
                    # yc tiles on GPSIMD; ne tiles land in wpool for stage 2
                    yc = [tt("yc0", e0[:], aa0[:], "+"),
                          tt("yc1", e1[:], aa1[:], "+"),
                          tt("yc2", e0[:], aa0[:], "-"),
                          tt("yc3", e1[:], aa1[:], "-")]
                    for spc in range(4):
                        yp = wpool.tile([P, NW], F16, name=f"yp_{mi}_{spc}")
                        ym = wpool.tile([P, NW], F16, name=f"ym_{mi}_{spc}")
                        nc.vector.tensor_add(yp[:], yc[spc][:], ne[spc][:])
                        nc.vector.tensor_sub(ym[:], yc[spc][:], ne[spc][:])
                        yy[0][g][kt][spc] = yp
                        yy[1][g][kt][spc] = ym

                return phase_e, phase_r

            phases = [m_phases(mi) for mi in range(8)]
            evs = [None] * 8
            evs[0] = phases[0][0]()
            for mi in range(1, 8):
                evs[mi] = phases[mi][0]()
                phases[mi - 1][1](evs[mi - 1])
            phases[7][1](evs[7])

            # ---- stage 2 ----
            # 16 pairs (spc, ss); each produces out rows srow (yp) and
            # srow+2048 (ym), all 1024 H columns.  c-side chain banks pack
            # [cyclic-256 | nega-256]; n-side is the direct nega-512.
            dcc = [0, 128]            # cyclic-256 window offsets, kt=0,1
            dcn = [0, 384]            # nega-256 window offsets, kt=2,3
            dhn = [(-k * P) % 1024 for k in range(4)]
            pairs = [(spc, ss) for spc in range(4) for ss in range(4)]
            cc_ev = [None] * 16

            def cc_chains(i):
                """c-side chains on uu/uv H-cols of yp/ym (group 0) + evac."""
                spc, ss = pairs[i]
                ssl = slice(ss * P, (ss + 1) * P)
                tg = ("epr", "c3") if i % 2 == 0 else ("epi", "l3n")
                res = []
                for side, nm in ((0, "cp"), (1, "cm")):
                    ps = ppool.tile([P, NW], F32, tag=tg[side],
                                    name=f"p_{nm}_{i}")
                    for k, kt in enumerate((0, 1)):
                        nc.tensor.matmul(ps[:, 0:HW2],
                                         yy[side][0][kt][spc][:, ssl],
                                         win["rot_hcc"][:, dcc[k]:dcc[k] + HW2],
                                         start=(k == 0), stop=(k == 1))
                    for k, kt in enumerate((2, 3)):
                        nc.tensor.matmul(ps[:, HW2:NW],
                                         yy[side][0][kt][spc][:, ssl],
                                         win["rot_hcn"][:, dcn[k]:dcn[k] + HW2],
                                         start=(k == 0), stop=(k == 1))
                    t = iopool.tile([P, NW], F16, tag=f"{nm}e", bufs=3,
                                    name=f"{nm}e_{i}")
                    nc.scalar.mul(t[:], ps[:], 1.0)
                    res.append(t)
                cc_ev[i] = res

            def nb_block(i):
                """n-side chains + un-CRT + finals + output DMA for pair i."""
                spc, ss = pairs[i]
                ssl = slice(ss * P, (ss + 1) * P)
                tg = ("emr", "a0") if i % 2 == 0 else ("emi", "a1")
                ev = []
                for side, nm in ((0, "np"), (1, "nm")):
                    ps = ppool.tile([P, NW], F32, tag=tg[side],
                                    name=f"p_{nm}_{i}")
                    for kt in range(4):
                        nc.tensor.matmul(ps[:], yy[side][1][kt][spc][:, ssl],
                                         win["rot_hn"][:, dhn[kt]:dhn[kt] + NW],
                                         start=(kt == 0), stop=(kt == 3))
                    t = iopool.tile([P, NW], F16, tag=f"{nm}e", bufs=2,
                                    name=f"{nm}e_{i}")
                    nc.scalar.mul(t[:], ps[:], 1.0)
                    ev.append(t)
                t_np, t_nm = ev
                t_cp, t_cm = cc_ev[i]
                # zc un-CRT (c256/n256 -> c512), obp flushed before zm
                zp = iopool.tile([P, NW], F16, tag="zp", bufs=2, name=f"zp_{i}")
                zm = iopool.tile([P, NW], F16, tag="zm", bufs=2, name=f"zm_{i}")
                srow = spc * NW + ss * P
                obp = iopool.tile([P, H], F16, tag="obp", bufs=2, name=f"obp_{i}")
                obm = iopool.tile([P, H], F16, tag="obm", bufs=2, name=f"obm_{i}")
                nc.vector.tensor_add(zp[:, 0:HW2], t_cp[:, 0:HW2], t_cp[:, HW2:])
                nc.vector.tensor_sub(zp[:, HW2:NW], t_cp[:, 0:HW2], t_cp[:, HW2:])
                nc.vector.tensor_add(obp[:, 0:NW], zp[:], t_np[:])
                nc.vector.tensor_sub(obp[:, NW:H], zp[:], t_np[:])
                nc.sync.dma_start(out[srow:srow + P, :], obp[:])
                nc.vector.tensor_add(zm[:, 0:HW2], t_cm[:, 0:HW2], t_cm[:, HW2:])
                nc.vector.tensor_sub(zm[:, HW2:NW], t_cm[:, 0:HW2], t_cm[:, HW2:])
                nc.vector.tensor_add(obm[:, 0:NW], zm[:], t_nm[:])
                nc.vector.tensor_sub(obm[:, NW:H], zm[:], t_nm[:])
                nc.sync.dma_start(out[MS + srow:MS + srow + P, :], obm[:])

            LA = 2
            for i in range(16):
                cc_chains(i)
                if i >= LA:
                    nb_block(i - LA)
            for i in range(16 - LA, 16):
                nb_block(i)

    nc.compile()
    return nc


def _rot(vec, width):
    p = np.arange(P)[:, None]
    mod = len(vec)
    return vec[(np.arange(width)[None, :] - p) % mod].astype(np.float16)


def _prep_windows(seq_circ, hidden_circ):
    beta = np.exp(1j * np.pi / 4)
    cs = seq_circ.astype(np.float64)
    cp = 0.5 * (cs[:MS] + cs[MS:])
    cn = 0.5 * (cs[:MS] - cs[MS:])
    cpp = 0.5 * (cp[:1024] + cp[1024:])
    cpn = 0.5 * (cp[:1024] - cp[1024:])
    cppp = 0.5 * (cpp[:512] + cpp[512:])
    cpn3 = 0.5 * (cpp[:512] - cpp[512:])
    bc = cn[:1024] + 1j * cn[1024:]
    bp = 0.5 * (bc[:512] + beta * bc[512:])
    bm = 0.5 * (bc[:512] - beta * bc[512:])
    bext_p = np.concatenate([bp, beta * bp])
    bext_m = np.concatenate([bm, -beta * bm])
    bc2 = cpn[:512] + 1j * cpn[512:]
    b2p = 0.5 * (bc2[:256] + beta * bc2[256:])
    b2m = 0.5 * (bc2[:256] - beta * bc2[256:])
    bext2p = np.concatenate([b2p, beta * b2p])
    bext2m = np.concatenate([b2m, -beta * b2m])
    ch = hidden_circ.astype(np.float64)
    hp = 0.5 * (ch[:MH] + ch[MH:])
    hn = 0.5 * (ch[:MH] - ch[MH:])
    hp2p = 0.5 * (hp[:256] + hp[256:])
    hp2n = 0.5 * (hp[:256] - hp[256:])
    vecs = {
        "w_tpr": bext_p.real, "w_tpi": bext_p.imag,
        "w_tmr": bext_m.real, "w_tmi": bext_m.imag,
        "w2pr": bext2p.real, "w2pi": bext2p.imag,
        "w2mr": bext2m.real, "w2mi": bext2m.imag,
        "rot_ccc": cppp, "rot_ccn": np.concatenate([cpn3, -cpn3]),
        "rot_hcc": hp2p, "rot_hcn": np.concatenate([hp2n, -hp2n]),
        "rot_hn": np.concatenate([hn, -hn]),
    }
    packed = np.concatenate([_rot(vecs[n], w) for n, w in WIN_LAYOUT], axis=1)
    return np.ascontiguousarray(packed)


def _fold_tree(G):
    """G: [B, 4096, C] fp32 -> packed operand chunks [B, 32, 128, C]."""
    u = G[:, :MS] + G[:, MS:]
    v = G[:, :MS] - G[:, MS:]
    u2 = u[:, :1024] + u[:, 1024:]
    v2 = u[:, :1024] - u[:, 1024:]
    u3 = u2[:, :512] + u2[:, 512:]
    v3 = u2[:, :512] - u2[:, 512:]
    cre, cim = v[:, :1024], v[:, 1024:]
    t1 = SQ * (cre[:, 512:] - cim[:, 512:])
    t2 = SQ * (cre[:, 512:] + cim[:, 512:])
    c2re, c2im = v2[:, :512], v2[:, 512:]
    s1 = SQ * (c2re[:, 256:] - c2im[:, 256:])
    s2 = SQ * (c2re[:, 256:] + c2im[:, 256:])
    blocks = np.concatenate([
        cre[:, :512] + t1, cim[:, :512] + t2,
        cre[:, :512] - t1, cim[:, :512] - t2,
        c2re[:, :256] + s1, c2im[:, :256] + s2,
        c2re[:, :256] - s1, c2im[:, :256] - s2,
        u3, v3,
    ], axis=1)
    bb = blocks.shape[0]
    return blocks.reshape(bb, 32, P, blocks.shape[2])


def _prep_ops(input_emb):
    x = np.asarray(input_emb, dtype=np.float32)
    uH = x[:, :, :MH] + x[:, :, MH:]
    vH = x[:, :, :MH] - x[:, :, MH:]
    # v2: group 0 carries the H-cyclic-512 CRT split (uu|uv)
    uu = uH[:, :, :HW2] + uH[:, :, HW2:]
    uv = uH[:, :, :HW2] - uH[:, :, HW2:]
    g0 = np.concatenate([uu, uv], axis=2)
    res = []
    for G in (g0, vH):
        t = _fold_tree(G).astype(np.float16)          # [B, 32, 128, 512]
        t = t.transpose(0, 2, 1, 3).reshape(B, P, 32 * NW)
        res.append(np.ascontiguousarray(t))
    return res


def _run(input_emb, seq_circ, hidden_circ, trace=False):
    if "nc" not in _CACHE:
        _CACHE["nc"] = _build()
    nc = _CACHE["nc"]
    wins = _prep_windows(np.asarray(seq_circ), np.asarray(hidden_circ))
    op0, op1 = _prep_ops(input_emb)
    in_maps = [{"op0": op0[b], "op1": op1[b], "wins": wins}
               for b in range(B)]
    res = bass_utils.run_bass_kernel_spmd(nc, in_maps, core_ids=list(range(B)),
                                          trace=trace)
    outp = np.stack([res.results[b]["out"] for b in range(B)])
    return outp.astype(np.float32), res


def kernel(input_emb, seq_circ, hidden_circ):
    outp, _ = _run(input_emb, seq_circ, hidden_circ, trace=False)
    return outp


# revision 24
# speedup vs baseline: 1.0226x; 1.0226x over previous
"""Trainium2 Bass kernel for the 2D circulant transform.

Math: per example b,  out[b] = C_s @ inp[b] @ C_h^T  where C_s/C_h are the
circulant matrices of seq_circ (S=4096) and hidden_circ (H=1024).
Data-parallel over batch: core b handles example b (B == 8 cores).

Stage 1 (S-axis, per 128-col block of H): CRT tree of C_4096 into
cyclic-512 + nega-512 (direct), nega-1024 (two twisted-256 complex
products), nega-2048 (two twisted-512 complex products); all folds are
elementwise on the input and precomputed on host.  fp16 operands/
windows, fp32 PSUM, ScalarE evacuates, VectorE recombines.

Stage 2 (H-axis): C_1024 = cyclic-512 + nega-512 along H.  The
cyclic-512 is further CRT-split into cyclic-256 + nega-256 -- its
operand fold is linear in input H-columns, commutes with the S
transform, and is done for FREE on the host (group-0 operand ships as
[uu|uv]).  c-side chains contract 256 cols into a [c256|n256] PSUM
bank; un-CRT + final H-recombine run on VectorE, output DMA'd fp16 and
upcast on host.

Engine notes (measured on HW):
- GPSIMD tensor ops cost ~1.25us per [128,512] (3.7x DVE) with multi-us
  library-reload warmups, and concurrently-running GPSIMD ops slow DVE
  ops ~4x via SBUF contention -- GPSIMD does DMA issue only.
- Windows that are pure sign flips (w_tpn=-w_tpi, ...) or antiperiodic
  tails (rot_ccn/rot_hcn/rot_hn: w[:,f+M] = -w[:,f]) are generated on
  DVE instead of DMA'd (-1.3MB off the bandwidth-bound startup stream).
- Output DMAs go on the idle sync queue; putting them on the scalar
  queue stalls ScalarE's evacuation stream (head-of-line blocking).
"""
import os
import sys

for _p in ("/opt/trn_rl_repo",):
    if _p not in sys.path and os.path.isdir(_p):
        sys.path.append(_p)

import numpy as np

import concourse.bacc as bacc
import concourse.mybir as mybir
import concourse.tile as tile
from concourse import bass_utils

B, S, H = 8, 4096, 1024
MS, MH = S // 2, H // 2
P = 128
NW = 512
HW2 = 256
F16 = mybir.dt.float16
F32 = mybir.dt.float32
SQ = float(np.sqrt(0.5))

_CACHE = {}

# DMA'd window layout: (name, width).  Sign-flipped partners (w_tpn,
# w_tmn, w2pn, w2mn) are generated on-device from these.  Nega windows
# (rot_ccn/rot_hcn/rot_hn) are antiperiodic (w[:,f+M] = -w[:,f]) so only
# the leading columns ship; the tail is generated on DVE.
WIN_LAYOUT = (
    ("w_tpr", 1408),                                     # seg 0 (first need)
    ("w_tpi", 1408),                                     # seg 1
    ("w_tmr", 1408), ("w_tmi", 1408),                    # seg 2
    ("w2pr", 640), ("w2pi", 640), ("w2mr", 640), ("w2mi", 640),  # seg 3
    ("rot_ccc", 896),                                    # seg 4
    ("rot_hcc", 384),                                    # seg 5
    ("rot_ccn", 896), ("rot_hcn", 384), ("rot_hn", 896),  # anti, own DMAs
)
SEG_SPLIT = (1408, 1408, 2816, 2560, 896, 384)
WIN_TOTAL = sum(w for _, w in WIN_LAYOUT)
GEN_WINS = (("w_tpn", "w_tpi", 1408), ("w_tmn", "w_tmi", 1408),
            ("w2pn", "w2pi", 640), ("w2mn", "w2mi", 640))
# name -> (full_width, shipped, gen_src_lo):  [shipped:full] = -[src:src+len]
ANTI_WINS = {"rot_ccn": (1408, 896, 384), "rot_hcn": (640, 384, 128),
             "rot_hn": (1408, 896, 384)}


def _build():
    nc = bacc.Bacc("TRN2", target_bir_lowering=False, debug=False,
                   num_devices=B)
    d_op = [nc.dram_tensor(f"op{g}", [P, 32 * NW], F16,
                           kind="ExternalInput").ap() for g in range(2)]
    d_win = nc.dram_tensor("wins", [P, WIN_TOTAL], F16,
                           kind="ExternalInput").ap()
    out = nc.dram_tensor("out", [S, H], F16, kind="ExternalOutput").ap()

    with tile.TileContext(nc) as tc:
        with tc.tile_pool(name="const", bufs=1) as cpool, \
             tc.tile_pool(name="work", bufs=1) as wpool, \
             tc.tile_pool(name="io", bufs=2) as iopool, \
             tc.tile_pool(name="ps", bufs=1, space="PSUM") as ppool:
            # seg tiles cover the non-antiperiodic windows contiguously;
            # antiperiodic windows get full-width tiles with partial DMA.
            win = {}
            win_off = {}
            off = 0
            for name, w in WIN_LAYOUT:
                win_off[name] = off
                off += w
            segs = []
            off = 0
            for w in SEG_SPLIT:
                segs.append((off, cpool.tile([P, w], F16, name=f"wseg{off}")))
                off += w
            off = 0
            for name, w in WIN_LAYOUT:
                if name in ANTI_WINS:
                    continue
                for so, st in segs:
                    if so <= off < so + st.shape[1]:
                        win[name] = st[:, off - so:off - so + w]
                        break
                off += w
            for name, (full, ship, src) in ANTI_WINS.items():
                win[name] = cpool.tile([P, full], F16, name=name)[:]
            # device-generated sign-flipped windows (DVE, idle at startup)
            for gname, src, w in GEN_WINS:
                t = cpool.tile([P, w], F16, name=gname)
                win[gname] = t[:]

            def anti_dma(name, eng):
                full, ship, src = ANTI_WINS[name]
                eng.dma_start(win[name][:, 0:ship],
                              d_win[:, win_off[name]:win_off[name] + ship])

            def anti_gen(name):
                full, ship, src = ANTI_WINS[name]
                nc.vector.tensor_scalar_mul(win[name][:, ship:full],
                                            win[name][:, src:src + full - ship],
                                            -1.0)

            opq = [[None] * 8 for _ in range(2)]
            opa = [None, None]
            opc = [None, None]

            def load_opq(g, q, eng):
                for h in range(2):
                    qq = 2 * q + h
                    t = iopool.tile([P, 2 * NW], F16, tag=f"opq{qq}", bufs=2,
                                    name=f"opq{qq}_{g}")
                    eng.dma_start(t[:], d_op[g][:, qq * 2 * NW:
                                                (qq + 1) * 2 * NW])
                    opq[g][qq] = t

            def load_ac_a(g, eng):
                opa[g] = iopool.tile([P, 8 * NW], F16, tag="opa", bufs=1,
                                     name=f"opa_{g}")
                eng.dma_start(opa[g][:], d_op[g][:, 16 * NW:24 * NW])

            def load_ac_c(g, eng):
                opc[g] = iopool.tile([P, 8 * NW], F16, tag="opc", bufs=1,
                                     name=f"opc_{g}")
                eng.dma_start(opc[g][:], d_op[g][:, 24 * NW:32 * NW])

            def seg_dma(eng, i):
                lo = sum(SEG_SPLIT[:i])
                eng.dma_start(segs[i][1][:], d_win[:, lo:lo + SEG_SPLIT[i]])

            # deadline-ordered bulk DMAs across the three DGE queues.
            # gpsimd's SWDGE (~60GB/s) carries only the late-need windows.
            seg_dma(nc.sync, 0)           # w_tpr (first need, alone)
            load_opq(0, 0, nc.scalar)     # d+re
            seg_dma(nc.scalar, 1)         # w_tpi
            load_opq(0, 1, nc.sync)       # d+im
            seg_dma(nc.sync, 2)           # w_tmr | w_tmi
            load_opq(0, 2, nc.scalar)     # d-re
            seg_dma(nc.gpsimd, 4)         # rot_ccc
            anti_dma("rot_ccn", nc.gpsimd)
            seg_dma(nc.scalar, 3)         # w2*
            load_opq(0, 3, nc.sync)       # d-im
            load_ac_a(0, nc.scalar)
            load_ac_c(0, nc.scalar)
            load_opq(1, 0, nc.sync)
            load_opq(1, 1, nc.scalar)
            load_opq(1, 2, nc.sync)
            load_opq(1, 3, nc.scalar)
            seg_dma(nc.gpsimd, 5)         # rot_hcc
            anti_dma("rot_hcn", nc.gpsimd)
            anti_dma("rot_hn", nc.gpsimd)
            load_ac_a(1, nc.sync)
            load_ac_c(1, nc.sync)

            # DVE window gens, ordered by first need
            gen = {g[0]: g for g in GEN_WINS}
            for gname, src, w in (gen["w_tpn"], gen["w_tmn"]):
                nc.vector.tensor_scalar_mul(win[gname], win[src], -1.0)
            anti_gen("rot_ccn")
            for gname, src, w in (gen["w2pn"], gen["w2mn"]):
                nc.vector.tensor_scalar_mul(win[gname], win[src], -1.0)
            anti_gen("rot_hcn")
            anti_gen("rot_hn")

            # stage-1 outputs: yy[side][g][kt][spc], side 0 = yp
            yy = [[[[None] * 4 for _ in range(4)] for _ in range(2)]
                  for _ in range(2)]

            def m_phases(mi):
                g, kt = mi // 4, mi % 4

                def osl(i):
                    if i < 16:
                        buf, j = opq[g][i // 2], i % 2
                    elif i < 24:
                        buf, j = opa[g], i - 16
                    else:
                        buf, j = opc[g], i - 24
                    c0 = j * NW + kt * P
                    return buf[:, c0:c0 + P]

                def chain(tag, mms, n_w=NW):
                    ps = ppool.tile([P, NW], F32, tag=tag, name=f"p_{tag}_{mi}")
                    n = len(mms)
                    for i, (plo, o, wname, d) in enumerate(mms):
                        nc.tensor.matmul(ps[:, plo:plo + n_w], osl(o),
                                         win[wname][:, d:d + n_w],
                                         start=(i == 0), stop=(i == n - 1))
                    return ps

                def evac(name, ps):
                    t = iopool.tile([P, NW], F16, tag=f"{name}e", bufs=2,
                                    name=f"{name}e_{mi}")
                    nc.scalar.mul(t[:], ps[:], 1.0)
                    return t

                def tt(name, a, b, op_, bufs=1, wid=NW, eng=None):
                    t = iopool.tile([P, wid], F16, tag=name, bufs=bufs,
                                    name=f"{name}_{mi}")
                    eng = eng or nc.vector
                    (eng.tensor_add if op_ == "+" else eng.tensor_sub)(
                        t[:], a, b)
                    return t

                dtw = [(-j * P) % 1024 for j in range(4)]

                def phase_e():
                    p_epr = chain("epr",
                                  [(0, j, "w_tpr", dtw[j]) for j in range(4)]
                                  + [(0, 4 + j, "w_tpn", dtw[j]) for j in range(4)])
                    p_epi = chain("epi",
                                  [(0, j, "w_tpi", dtw[j]) for j in range(4)]
                                  + [(0, 4 + j, "w_tpr", dtw[j]) for j in range(4)])
                    e_pr = evac("epr", p_epr)
                    e_pi = evac("epi", p_epi)
                    p_emr = chain("emr",
                                  [(0, 8 + j, "w_tmr", dtw[j]) for j in range(4)]
                                  + [(0, 12 + j, "w_tmn", dtw[j]) for j in range(4)])
                    p_emi = chain("emi",
                                  [(0, 8 + j, "w_tmi", dtw[j]) for j in range(4)]
                                  + [(0, 12 + j, "w_tmr", dtw[j]) for j in range(4)])
                    e_mr = evac("emr", p_emr)
                    e_mi = evac("emi", p_emi)
                    return e_pr, e_pi, e_mr, e_mi

                def phase_r(ev):
                    e_pr, e_pi, e_mr, e_mi = ev
                    ne = [None] * 4
                    ne[0] = tt("ne0", e_pr[:], e_mr[:], "+")
                    ne[2] = tt("ne2", e_pi[:], e_mi[:], "+")
                    dre = tt("dre", e_pr[:], e_mr[:], "-")
                    dim = tt("dim", e_pi[:], e_mi[:], "-")
                    t3 = tt("t3", dre[:], dim[:], "+")
                    t4 = tt("t4", dim[:], dre[:], "-")
                    ne[1] = iopool.tile([P, NW], F16, tag="ne1", bufs=1,
                                        name=f"ne1_{mi}")
                    nc.vector.tensor_scalar_mul(ne[1][:], t3[:], SQ)
                    ne[3] = iopool.tile([P, NW], F16, tag="ne3", bufs=1,
                                        name=f"ne3_{mi}")
                    nc.vector.tensor_scalar_mul(ne[3][:], t4[:], SQ)

                    d2 = (0, 384)
                    p_a0 = chain("a0",
                                 [(0, 16 + j, "w2pr", d2[j]) for j in range(2)]
                                 + [(0, 18 + j, "w2pn", d2[j]) for j in range(2)]
                                 + [(HW2, 20 + j, "w2mr", d2[j]) for j in range(2)]
                                 + [(HW2, 22 + j, "w2mn", d2[j]) for j in range(2)],
                                 n_w=HW2)
                    p_a1 = chain("a1",
                                 [(0, 16 + j, "w2pi", d2[j]) for j in range(2)]
                                 + [(0, 18 + j, "w2pr", d2[j]) for j in range(2)]
                                 + [(HW2, 20 + j, "w2mi", d2[j]) for j in range(2)]
                                 + [(HW2, 22 + j, "w2mr", d2[j]) for j in range(2)],
                                 n_w=HW2)
                    ar0 = evac("a0", p_a0)
                    ar1 = evac("a1", p_a1)
                    aa0 = iopool.tile([P, NW], F16, tag="a0e2", bufs=2,
                                      name=f"a0e2_{mi}")
                    aa1 = iopool.tile([P, NW], F16, tag="a1e2", bufs=2,
                                      name=f"a1e2_{mi}")
                    nc.vector.tensor_add(aa0[:, :HW2], ar0[:, :HW2],
                                         ar0[:, HW2:])
                    nc.vector.tensor_add(aa1[:, :HW2], ar1[:, :HW2],
                                         ar1[:, HW2:])
                    ddre = tt("ddre", ar0[:, :HW2], ar0[:, HW2:], "-", wid=HW2)
                    ddim = tt("ddim", ar1[:, :HW2], ar1[:, HW2:], "-", wid=HW2)
                    t3p = tt("t3p", ddre[:], ddim[:], "+", wid=HW2)
                    t4p = tt("t4p", ddim[:], ddre[:], "-", wid=HW2)
                    nc.vector.tensor_scalar_mul(aa0[:, HW2:], t3p[:], SQ)
                    nc.vector.tensor_scalar_mul(aa1[:, HW2:], t4p[:], SQ)

                    p_c3 = chain("c3", [(0, 24 + k, "rot_ccc", (-k * P) % 512)
                                        for k in range(4)])
                    p_n3 = chain("l3n", [(0, 28 + k, "rot_ccn",
                                          (-k * P) % 1024) for k in range(4)])
                    c3e = evac("c3", p_c3)
                    n3e = evac("l3n", p_n3)

                    e0 = tt("e0", c3e[:], n3e[:], "+")
                    e1 = tt("e1", c3e[:], n3e[:], "-")
                    # yc tiles on GPSIMD; ne tiles land in wpool for stage 2
                    yc = [tt("yc0", e0[:], aa0[:], "+"),
                          tt("yc1", e1[:], aa1[:], "+"),
                          tt("yc2", e0[:], aa0[:], "-"),
                          tt("yc3", e1[:], aa1[:], "-")]
                    for spc in range(4):
                        yp = wpool.tile([P, NW], F16, name=f"yp_{mi}_{spc}")
                        ym = wpool.tile([P, NW], F16, name=f"ym_{mi}_{spc}")
                        nc.vector.tensor_add(yp[:], yc[spc][:], ne[spc][:])
                        nc.vector.tensor_sub(ym[:], yc[spc][:], ne[spc][:])
                        yy[0][g][kt][spc] = yp
                        yy[1][g][kt][spc] = ym

                return phase_e, phase_r

            phases = [m_phases(mi) for mi in range(8)]
            evs = [None] * 8
            evs[0] = phases[0][0]()
            for mi in range(1, 8):
                evs[mi] = phases[mi][0]()
                phases[mi - 1][1](evs[mi - 1])
            phases[7][1](evs[7])

            # ---- stage 2 ----
            # 16 pairs (spc, ss); each produces out rows srow (yp) and
            # srow+2048 (ym), all 1024 H columns.  c-side chain banks pack
            # [cyclic-256 | nega-256]; n-side is the direct nega-512.
            dcc = [0, 128]            # cyclic-256 window offsets, kt=0,1
            dcn = [0, 384]            # nega-256 window offsets, kt=2,3
            dhn = [(-k * P) % 1024 for k in range(4)]
            pairs = [(spc, ss) for spc in range(4) for ss in range(4)]
            cc_ev = [None] * 16

            def cc_chains(i):
                """c-side chains on uu/uv H-cols of yp/ym (group 0) + evac."""
                spc, ss = pairs[i]
                ssl = slice(ss * P, (ss + 1) * P)
                tg = ("epr", "c3") if i % 2 == 0 else ("epi", "l3n")
                res = []
                for side, nm in ((0, "cp"), (1, "cm")):
                    ps = ppool.tile([P, NW], F32, tag=tg[side],
                                    name=f"p_{nm}_{i}")
                    for k, kt in enumerate((0, 1)):
                        nc.tensor.matmul(ps[:, 0:HW2],
                                         yy[side][0][kt][spc][:, ssl],
                                         win["rot_hcc"][:, dcc[k]:dcc[k] + HW2],
                                         start=(k == 0), stop=(k == 1))
                    for k, kt in enumerate((2, 3)):
                        nc.tensor.matmul(ps[:, HW2:NW],
                                         yy[side][0][kt][spc][:, ssl],
                                         win["rot_hcn"][:, dcn[k]:dcn[k] + HW2],
                                         start=(k == 0), stop=(k == 1))
                    t = iopool.tile([P, NW], F16, tag=f"{nm}e", bufs=4,
                                    name=f"{nm}e_{i}")
                    nc.scalar.mul(t[:], ps[:], 1.0)
                    res.append(t)
                cc_ev[i] = res

            def nb_block(i):
                """n-side chains + un-CRT + finals + output DMA for pair i."""
                spc, ss = pairs[i]
                ssl = slice(ss * P, (ss + 1) * P)
                tg = ("emr", "a0") if i % 2 == 0 else ("emi", "a1")
                ev = []
                for side, nm in ((0, "np"), (1, "nm")):
                    ps = ppool.tile([P, NW], F32, tag=tg[side],
                                    name=f"p_{nm}_{i}")
                    for kt in range(4):
                        nc.tensor.matmul(ps[:], yy[side][1][kt][spc][:, ssl],
                                         win["rot_hn"][:, dhn[kt]:dhn[kt] + NW],
                                         start=(kt == 0), stop=(kt == 3))
                    t = iopool.tile([P, NW], F16, tag=f"{nm}e", bufs=2,
                                    name=f"{nm}e_{i}")
                    nc.scalar.mul(t[:], ps[:], 1.0)
                    ev.append(t)
                t_np, t_nm = ev
                t_cp, t_cm = cc_ev[i]
                # zc un-CRT (c256/n256 -> c512), obp flushed before zm
                zp = iopool.tile([P, NW], F16, tag="zp", bufs=2, name=f"zp_{i}")
                zm = iopool.tile([P, NW], F16, tag="zm", bufs=2, name=f"zm_{i}")
                srow = spc * NW + ss * P
                obp = iopool.tile([P, H], F16, tag="obp", bufs=2, name=f"obp_{i}")
                obm = iopool.tile([P, H], F16, tag="obm", bufs=2, name=f"obm_{i}")
                nc.vector.tensor_add(zp[:, 0:HW2], t_cp[:, 0:HW2], t_cp[:, HW2:])
                nc.vector.tensor_sub(zp[:, HW2:NW], t_cp[:, 0:HW2], t_cp[:, HW2:])
                nc.vector.tensor_add(obp[:, 0:NW], zp[:], t_np[:])
                nc.vector.tensor_sub(obp[:, NW:H], zp[:], t_np[:])
                nc.sync.dma_start(out[srow:srow + P, :], obp[:])
                nc.vector.tensor_add(zm[:, 0:HW2], t_cm[:, 0:HW2], t_cm[:, HW2:])
                nc.vector.tensor_sub(zm[:, HW2:NW], t_cm[:, 0:HW2], t_cm[:, HW2:])
                nc.vector.tensor_add(obm[:, 0:NW], zm[:], t_nm[:])
                nc.vector.tensor_sub(obm[:, NW:H], zm[:], t_nm[:])
                nc.sync.dma_start(out[MS + srow:MS + srow + P, :], obm[:])

            LA = 3
            for i in range(16):
                cc_chains(i)
                if i >= LA:
                    nb_block(i - LA)
            for i in range(16 - LA, 16):
                nb_block(i)

    nc.compile()
    return nc


def _rot(vec, width):
    p = np.arange(P)[:, None]
    mod = len(vec)
    return vec[(np.arange(width)[None, :] - p) % mod].astype(np.float16)


def _prep_windows(seq_circ, hidden_circ):
    beta = np.exp(1j * np.pi / 4)
    cs = seq_circ.astype(np.float64)
    cp = 0.5 * (cs[:MS] + cs[MS:])
    cn = 0.5 * (cs[:MS] - cs[MS:])
    cpp = 0.5 * (cp[:1024] + cp[1024:])
    cpn = 0.5 * (cp[:1024] - cp[1024:])
    cppp = 0.5 * (cpp[:512] + cpp[512:])
    cpn3 = 0.5 * (cpp[:512] - cpp[512:])
    bc = cn[:1024] + 1j * cn[1024:]
    bp = 0.5 * (bc[:512] + beta * bc[512:])
    bm = 0.5 * (bc[:512] - beta * bc[512:])
    bext_p = np.concatenate([bp, beta * bp])
    bext_m = np.concatenate([bm, -beta * bm])
    bc2 = cpn[:512] + 1j * cpn[512:]
    b2p = 0.5 * (bc2[:256] + beta * bc2[256:])
    b2m = 0.5 * (bc2[:256] - beta * bc2[256:])
    bext2p = np.concatenate([b2p, beta * b2p])
    bext2m = np.concatenate([b2m, -beta * b2m])
    ch = hidden_circ.astype(np.float64)
    hp = 0.5 * (ch[:MH] + ch[MH:])
    hn = 0.5 * (ch[:MH] - ch[MH:])
    hp2p = 0.5 * (hp[:256] + hp[256:])
    hp2n = 0.5 * (hp[:256] - hp[256:])
    vecs = {
        "w_tpr": bext_p.real, "w_tpi": bext_p.imag,
        "w_tmr": bext_m.real, "w_tmi": bext_m.imag,
        "w2pr": bext2p.real, "w2pi": bext2p.imag,
        "w2mr": bext2m.real, "w2mi": bext2m.imag,
        "rot_ccc": cppp, "rot_ccn": np.concatenate([cpn3, -cpn3]),
        "rot_hcc": hp2p, "rot_hcn": np.concatenate([hp2n, -hp2n]),
        "rot_hn": np.concatenate([hn, -hn]),
    }
    packed = np.concatenate([_rot(vecs[n], w) for n, w in WIN_LAYOUT], axis=1)
    return np.ascontiguousarray(packed)


def _fold_tree(G):
    """G: [B, 4096, C] fp32 -> packed operand chunks [B, 32, 128, C]."""
    u = G[:, :MS] + G[:, MS:]
    v = G[:, :MS] - G[:, MS:]
    u2 = u[:, :1024] + u[:, 1024:]
    v2 = u[:, :1024] - u[:, 1024:]
    u3 = u2[:, :512] + u2[:, 512:]
    v3 = u2[:, :512] - u2[:, 512:]
    cre, cim = v[:, :1024], v[:, 1024:]
    t1 = SQ * (cre[:, 512:] - cim[:, 512:])
    t2 = SQ * (cre[:, 512:] + cim[:, 512:])
    c2re, c2im = v2[:, :512], v2[:, 512:]
    s1 = SQ * (c2re[:, 256:] - c2im[:, 256:])
    s2 = SQ * (c2re[:, 256:] + c2im[:, 256:])
    blocks = np.concatenate([
        cre[:, :512] + t1, cim[:, :512] + t2,
        cre[:, :512] - t1, cim[:, :512] - t2,
        c2re[:, :256] + s1, c2im[:, :256] + s2,
        c2re[:, :256] - s1, c2im[:, :256] - s2,
        u3, v3,
    ], axis=1)
    bb = blocks.shape[0]
    return blocks.reshape(bb, 32, P, blocks.shape[2])


def _prep_ops(input_emb):
    x = np.asarray(input_emb, dtype=np.float32)
    uH = x[:, :, :MH] + x[:, :, MH:]
    vH = x[:, :, :MH] - x[:, :, MH:]
    # v2: group 0 carries the H-cyclic-512 CRT split (uu|uv)
    uu = uH[:, :, :HW2] + uH[:, :, HW2:]
    uv = uH[:, :, :HW2] - uH[:, :, HW2:]
    g0 = np.concatenate([uu, uv], axis=2)
    res = []
    for G in (g0, vH):
        t = _fold_tree(G).astype(np.float16)          # [B, 32, 128, 512]
        t = t.transpose(0, 2, 1, 3).reshape(B, P, 32 * NW)
        res.append(np.ascontiguousarray(t))
    return res


def _run(input_emb, seq_circ, hidden_circ, trace=False):
    if "nc" not in _CACHE:
        _CACHE["nc"] = _build()
    nc = _CACHE["nc"]
    wins = _prep_windows(np.asarray(seq_circ), np.asarray(hidden_circ))
    op0, op1 = _prep_ops(input_emb)
    in_maps = [{"op0": op0[b], "op1": op1[b], "wins": wins}
               for b in range(B)]
    res = bass_utils.run_bass_kernel_spmd(nc, in_maps, core_ids=list(range(B)),
                                          trace=trace)
    outp = np.stack([res.results[b]["out"] for b in range(B)])
    return outp.astype(np.float32), res


def kernel(input_emb, seq_circ, hidden_circ):
    outp, _ = _run(input_emb, seq_circ, hidden_circ, trace=False)
    return outp
